# revision 28
# baseline (speedup 1.0000x reference)
"""GF(2) linear block encoder c = (b @ G) mod 2 on 8 TRN2 NeuronCores.

Strategy (measured ~135us, PE-bound at the fp8 peak):
  - Data-parallel: shard b rows (32768 -> 8 x 4096), replicate G.
  - Bits {0,1} are exact in fp8-e4m3 and products accumulate exactly in
    fp32 PSUM (sums <= 1024 << 2^24), so the GF(2) matmul runs as fp8
    DoubleRow matmuls (K=256 per MM) at the PE's rated fp8 peak
    (~110us/core; measured 216ns per 512-row MM back to back).
  - Counts are cast fp32 -> uint16 on chip (ACT low half / DVE high
    half in parallel; fp32->uint8 saturates at 255 on HW, and DVE
    bitwise ops cannot cast); the HOST extracts the parity bit during
    the int32 upcast, so the device ships 16MB/core instead of 32.
  - b is host-packed as [128, MT, KS, 128] (k = s*128 + p, m = mt*128+j)
    so every m-chunk DMA is contiguous per partition.
  - G heads both HWDGE queues (it gates the first m-tiles) with b0
    squeezed in front on one; dummy warmup matmuls on memset SBUF ramp
    the PE p-state during the input wait; m-tiles 0+1 interleave at
    kp-stage granularity to match the G arrival cadence; the last tile
    converts+DMAs in 512-col chunks so only 128KB trails the final MM.
  - gpsimd/SWDGE is never used for DMA (its end-of-kernel drain costs
    ~4us); outputs alternate sync/scalar, which are idle after inputs.
"""

import sys

import numpy as np

if "/opt/trn_rl_repo" not in sys.path:
    sys.path.insert(0, "/opt/trn_rl_repo")

import ml_dtypes

B_ROWS = 32768
K_MSG = 1024
N_CODE = 2048
NCORES = 8
M = B_ROWS // NCORES  # 4096 rows per core
KS = K_MSG // 128     # 8 k-subtiles of 128
KP = KS // 2          # 4 DoubleRow k-pair steps (K=256 each)
MT = M // 128         # 32 m-tiles
NT = N_CODE // 512    # 4 n-tiles

# b chunk sizes in m-tiles: small first chunks let the PE start early
B_CHUNKS = [2, 2, 4, 4, 4, 8, 8]
assert sum(B_CHUNKS) == MT

F8 = ml_dtypes.float8_e4m3

_NC_CACHE = None


def _build_bass():
    import concourse.bacc as bacc
    import concourse.mybir as mybir
    from concourse import tile

    nc = bacc.Bacc("TRN2", target_bir_lowering=False, debug=False)

    bt = nc.dram_tensor("bt", [128, MT, KS, 128], mybir.dt.float8e4, kind="ExternalInput")
    g = nc.dram_tensor("g", [128, KS, N_CODE], mybir.dt.float8e4, kind="ExternalInput")
    c = nc.dram_tensor("c", [M, N_CODE], mybir.dt.uint16, kind="ExternalOutput")

    dr = mybir.MatmulPerfMode.DoubleRow

    with tile.TileContext(nc) as tc:
        with (
            tc.tile_pool(name="persist", bufs=1) as persist,
            tc.tile_pool(name="psum", bufs=2, space="PSUM") as psum_pool,
            tc.tile_pool(name="c16s", bufs=4) as c16s,
        ):
            # input DMA schedule: first b m-tile, then G k-pair tiles split
            # across both HWDGE queues, then the rest of b — balanced so
            # each queue carries ~3MB and every tile lands well before the
            # PE consumes it
            b_tiles = []
            off = 0
            for w in B_CHUNKS:
                b_tiles.append(
                    (
                        off,
                        w,
                        persist.tile(
                            [128, w, KS, 128],
                            mybir.dt.float8e4,
                            name=f"b{len(b_tiles)}",
                        ),
                    )
                )
                off += w
            g_tiles = [
                persist.tile([128, 2, N_CODE], mybir.dt.float8e4, name=f"g{kp}")
                for kp in range(KP)
            ]

            def ld_b(i):
                o, w, t = b_tiles[i]
                return lambda e: e.dma_start(out=t, in_=bt[:, o : o + w])

            def ld_g(kp):
                return lambda e: e.dma_start(
                    out=g_tiles[kp], in_=g[:, 2 * kp : 2 * kp + 2, :]
                )

            sync_q = [ld_g(0), ld_g(2), ld_b(1), ld_b(3), ld_b(5)]
            scal_q = [ld_b(0), ld_g(1), ld_g(3), ld_b(2), ld_b(4), ld_b(6)]
            for f in sync_q:
                f(nc.sync)
            for f in scal_q:
                f(nc.scalar)

            # PE p-state warmup: ~14 dummy matmuls on memset SBUF run
            # during the input-DMA wait so the PE reaches full clock before
            # real data lands (the first ~3us of PE busy run at 0.65-1.2GHz)
            warm_b = persist.tile([128, 2, 128], mybir.dt.float8e4, name="warm_b")
            warm_g = persist.tile([128, 2, 512], mybir.dt.float8e4, name="warm_g")
            warm_ps = psum_pool.tile([128, N_CODE], mybir.dt.float32, name="ps")
            nc.gpsimd.memset(warm_b, 0.0)
            nc.gpsimd.memset(warm_g, 0.0)
            for _ in range(10):
                nc.tensor.matmul(
                    warm_ps[:, :512],
                    warm_b,
                    warm_g,
                    start=True,
                    stop=True,
                    perf_mode=dr,
                )

            # chunk index for each m-tile
            owner = []
            for ci, w in enumerate(B_CHUNKS):
                owner += [ci] * w

            c_view = c.rearrange("(mt p) n -> mt p n", p=128)

            H = N_CODE // 2

            def emit_mms(ps, btile, j, kp):
                stat = btile[:, j, 2 * kp : 2 * kp + 2, :]
                for nt in range(NT):
                    nc.tensor.matmul(
                        ps[:, nt * 512 : (nt + 1) * 512],
                        stat,
                        g_tiles[kp][:, :, nt * 512 : (nt + 1) * 512],
                        start=(kp == 0),
                        stop=(kp == KP - 1),
                        perf_mode=dr,
                    )

            # m-tiles 0+1 interleaved at kp-stage granularity: 8 matmuls per
            # arriving G k-pair tile (~1.7us apart) keep the PE fed while G
            # still streams in; both PSUM buffers are in flight
            ps01 = [
                psum_pool.tile([128, N_CODE], mybir.dt.float32, name="ps")
                for _ in range(2)
            ]
            for kp in range(KP):
                for m in range(2):
                    emit_mms(ps01[m], b_tiles[0][2], m, kp)

            for mt in range(MT):
                ci = owner[mt]
                o, w, btile = b_tiles[ci]
                j = mt - o
                c16 = c16s.tile([128, N_CODE], mybir.dt.uint16, name="c16")
                if mt < 2:
                    ps = ps01[mt]
                else:
                    ps = psum_pool.tile([128, N_CODE], mybir.dt.float32, name="ps")  # 4 banks
                    # kp-outer: tracks the G arrival order at the start
                    for kp in range(KP):
                        emit_mms(ps, btile, j, kp)
                # cast fp32 counts -> uint16 (exact, counts <= 1024;
                # fp32->uint8 would saturate at 255 on HW). The host extracts
                # the parity bit. ACT converts the low n-half while DVE does
                # the high half so the per-tile convert latency halves.
                if mt < MT - 1:
                    nc.scalar.activation(
                        c16[:, :H], ps[:, :H], mybir.ActivationFunctionType.Copy
                    )
                    nc.vector.tensor_scalar(
                        out=c16[:, H:],
                        in0=ps[:, H:],
                        scalar1=1.0,
                        scalar2=None,
                        op0=mybir.AluOpType.mult,
                    )
                else:
                    # last tile: convert + DMA in 512-col chunks, emission
                    # interleaved ACT/DVE so both engines start right after
                    # the final matmul; only one 128KB chunk trails
                    for q in (0, 2, 1, 3):
                        lo, hi = q * 512, (q + 1) * 512
                        if q < 2:
                            nc.scalar.activation(
                                c16[:, lo:hi],
                                ps[:, lo:hi],
                                mybir.ActivationFunctionType.Copy,
                            )
                        else:
                            nc.vector.tensor_scalar(
                                out=c16[:, lo:hi],
                                in0=ps[:, lo:hi],
                                scalar1=1.0,
                                scalar2=None,
                                op0=mybir.AluOpType.mult,
                            )
                        out_eng = (nc.sync, nc.scalar)[q % 2]
                        out_eng.dma_start(
                            out=c_view[mt][:, lo:hi], in_=c16[:, lo:hi]
                        )
                # outputs alternate the two HWDGE queues (idle once inputs
                # land); gpsimd/SWDGE is avoided — its end-of-kernel drain
                # costs ~4us
                if mt < MT - 1:
                    out_eng = (nc.sync, nc.scalar)[mt % 2]
                    out_eng.dma_start(out=c_view[mt], in_=c16)

    nc.finalize()  # bacc: regalloc + event-semaphore legalization
    return nc


def _get_nc():
    global _NC_CACHE
    if _NC_CACHE is None:
        _NC_CACHE = _build_bass()
    return _NC_CACHE


def _pack_inputs(b, G):
    b8 = np.asarray(b).astype(np.uint8)
    G8 = np.asarray(G).astype(np.uint8)
    # g: [p, s, n] with k = s*128 + p
    g_f8 = G8.reshape(KS, 128, N_CODE).transpose(1, 0, 2).astype(F8, order="C")
    bts = []
    for core in range(NCORES):
        sh = b8[core * M : (core + 1) * M]  # [M, K]
        # [p, mt, s, j]: k = s*128 + p, m = mt*128 + j
        bt = sh.reshape(MT, 128, KS, 128).transpose(3, 0, 2, 1)
        bts.append(bt.astype(F8, order="C"))
    return bts, g_f8


def kernel(b, G, trace=False, **run_kwargs):
    from concourse.bass_utils import run_bass_kernel_spmd

    nc = _get_nc()
    bts, g_f8 = _pack_inputs(b, G)
    in_maps = [{"bt": bts[i], "g": g_f8} for i in range(NCORES)]
    res = run_bass_kernel_spmd(
        nc, in_maps, core_ids=list(range(NCORES)), trace=trace, **run_kwargs
    )
    out = np.concatenate([res.results[i]["c"] for i in range(NCORES)], axis=0)
    out = (out & 1).astype(np.int32)
    if trace:
        kernel.last_results = res
    return out


kernel.last_results = None


# revision 35
# speedup vs baseline: 1.1764x; 1.1764x over previous
"""GF(2) linear block encoder c = (b @ G) mod 2 on 8 TRN2 NeuronCores.

Strategy (measured ~135us, PE-bound at the fp8 peak):
  - Data-parallel: shard b rows (32768 -> 8 x 4096), replicate G.
  - Bits {0,1} are exact in fp8-e4m3 and products accumulate exactly in
    fp32 PSUM (sums <= 1024 << 2^24), so the GF(2) matmul runs as fp8
    DoubleRow matmuls (K=256 per MM) at the PE's rated fp8 peak
    (~110us/core; measured 216ns per 512-row MM back to back).
  - Counts are cast fp32 -> uint16 on chip (ACT low half / DVE high
    half in parallel; fp32->uint8 saturates at 255 on HW, and DVE
    bitwise ops cannot cast); the HOST extracts the parity bit during
    the int32 upcast, so the device ships 16MB/core instead of 32.
  - b is host-packed as [128, MT, KS, 128] (k = s*128 + p, m = mt*128+j)
    so every m-chunk DMA is contiguous per partition.
  - G heads both HWDGE queues (it gates the first m-tiles) with b0
    squeezed in front on one; dummy warmup matmuls on memset SBUF ramp
    the PE p-state during the input wait; m-tiles 0+1 interleave at
    kp-stage granularity to match the G arrival cadence; the last tile
    converts+DMAs in 512-col chunks so only 128KB trails the final MM.
  - gpsimd/SWDGE is never used for DMA (its end-of-kernel drain costs
    ~4us); outputs alternate sync/scalar, which are idle after inputs.
"""

import sys

import numpy as np

if "/opt/trn_rl_repo" not in sys.path:
    sys.path.insert(0, "/opt/trn_rl_repo")

import ml_dtypes

B_ROWS = 32768
K_MSG = 1024
N_CODE = 2048
NCORES = 8
M = B_ROWS // NCORES  # 4096 rows per core
KS = K_MSG // 128     # 8 k-subtiles of 128
KP = KS // 2          # 4 DoubleRow k-pair steps (K=256 each)
MT = M // 128         # 32 m-tiles
NT = N_CODE // 512    # 4 n-tiles

# b chunk sizes in m-tiles: small first chunks let the PE start early
B_CHUNKS = [2, 2, 4, 4, 4, 8, 8]
assert sum(B_CHUNKS) == MT

F8 = ml_dtypes.float8_e4m3

_NC_CACHE = None


def _build_bass():
    import concourse.bacc as bacc
    import concourse.mybir as mybir
    from concourse import tile

    nc = bacc.Bacc("TRN2", target_bir_lowering=False, debug=False)

    bt = nc.dram_tensor("bt", [128, MT, KS, 128], mybir.dt.float8e4, kind="ExternalInput")
    g = nc.dram_tensor("g", [128, KS, N_CODE], mybir.dt.float8e4, kind="ExternalInput")
    c = nc.dram_tensor("c", [M, N_CODE], mybir.dt.uint16, kind="ExternalOutput")

    dr = mybir.MatmulPerfMode.DoubleRow

    with tile.TileContext(nc) as tc:
        with (
            tc.tile_pool(name="persist", bufs=1) as persist,
            tc.tile_pool(name="psum", bufs=2, space="PSUM") as psum_pool,
            tc.tile_pool(name="c16s", bufs=6) as c16s,
        ):
            # input DMA schedule: first b m-tile, then G k-pair tiles split
            # across both HWDGE queues, then the rest of b — balanced so
            # each queue carries ~3MB and every tile lands well before the
            # PE consumes it
            b_tiles = []
            off = 0
            for w in B_CHUNKS:
                b_tiles.append(
                    (
                        off,
                        w,
                        persist.tile(
                            [128, w, KS, 128],
                            mybir.dt.float8e4,
                            name=f"b{len(b_tiles)}",
                        ),
                    )
                )
                off += w
            g_tiles = [
                persist.tile([128, 2, N_CODE], mybir.dt.float8e4, name=f"g{kp}")
                for kp in range(KP)
            ]

            def ld_b(i):
                o, w, t = b_tiles[i]
                return lambda e: e.dma_start(out=t, in_=bt[:, o : o + w])

            def ld_g(kp):
                return lambda e: e.dma_start(
                    out=g_tiles[kp], in_=g[:, 2 * kp : 2 * kp + 2, :]
                )

            sync_q = [ld_g(0), ld_g(2), ld_b(1), ld_b(3), ld_b(5)]
            scal_q = [ld_b(0), ld_g(1), ld_g(3), ld_b(2), ld_b(4), ld_b(6)]
            for f in sync_q:
                f(nc.sync)
            for f in scal_q:
                f(nc.scalar)

            # PE p-state warmup: ~14 dummy matmuls on memset SBUF run
            # during the input-DMA wait so the PE reaches full clock before
            # real data lands (the first ~3us of PE busy run at 0.65-1.2GHz)
            warm_b = persist.tile([128, 2, 128], mybir.dt.float8e4, name="warm_b")
            warm_g = persist.tile([128, 2, 512], mybir.dt.float8e4, name="warm_g")
            warm_ps = psum_pool.tile([128, N_CODE], mybir.dt.float32, name="ps")
            # PE p-state ramp needs ~3us of CONTINUOUS busy (idle gaps
            # reset it). 9 x 512-row warmups end ~13us, safely past the
            # worst-case g0 arrival, so the real stream starts at full
            # clock; overshooting is cheaper than re-ramping after an idle.
            nc.vector.memset(warm_b, 0.0)
            nc.vector.memset(warm_g, 0.0)
            for _ in range(9):
                nc.tensor.matmul(
                    warm_ps[:, :512],
                    warm_b,
                    warm_g,
                    start=True,
                    stop=True,
                    perf_mode=dr,
                )

            # chunk index for each m-tile
            owner = []
            for ci, w in enumerate(B_CHUNKS):
                owner += [ci] * w

            c_view = c.rearrange("(mt p) n -> mt p n", p=128)

            H = N_CODE // 2

            def emit_mms(ps, btile, j, kp):
                stat = btile[:, j, 2 * kp : 2 * kp + 2, :]
                for nt in range(NT):
                    nc.tensor.matmul(
                        ps[:, nt * 512 : (nt + 1) * 512],
                        stat,
                        g_tiles[kp][:, :, nt * 512 : (nt + 1) * 512],
                        start=(kp == 0),
                        stop=(kp == KP - 1),
                        perf_mode=dr,
                    )

            # m-tiles 0+1 interleaved at kp-stage granularity: 8 matmuls per
            # arriving G k-pair tile (~1.7us apart) keep the PE fed while G
            # still streams in; both PSUM buffers are in flight
            ps01 = [
                psum_pool.tile([128, N_CODE], mybir.dt.float32, name="ps")
                for _ in range(2)
            ]
            for kp in range(KP):
                for m in range(2):
                    emit_mms(ps01[m], b_tiles[0][2], m, kp)

            for mt in range(MT):
                ci = owner[mt]
                o, w, btile = b_tiles[ci]
                j = mt - o
                c16 = c16s.tile([128, N_CODE], mybir.dt.uint16, name="c16")
                if mt < 2:
                    ps = ps01[mt]
                else:
                    ps = psum_pool.tile([128, N_CODE], mybir.dt.float32, name="ps")  # 4 banks
                    # kp-outer: tracks the G arrival order at the start
                    for kp in range(KP):
                        emit_mms(ps, btile, j, kp)
                # cast fp32 counts -> uint16 (exact, counts <= 1024;
                # fp32->uint8 would saturate at 255 on HW). The host extracts
                # the parity bit. ACT converts the low n-half while DVE does
                # the high half so the per-tile convert latency halves.
                if mt < MT - 1:
                    nc.scalar.activation(
                        c16[:, :H], ps[:, :H], mybir.ActivationFunctionType.Copy
                    )
                    nc.vector.tensor_scalar(
                        out=c16[:, H:],
                        in0=ps[:, H:],
                        scalar1=1.0,
                        scalar2=None,
                        op0=mybir.AluOpType.mult,
                    )
                else:
                    # last tile: convert + DMA in 512-col chunks, emission
                    # interleaved ACT/DVE so both engines start right after
                    # the final matmul; only one 128KB chunk trails
                    for q in (0, 2, 1, 3):
                        lo, hi = q * 512, (q + 1) * 512
                        if q < 2:
                            nc.scalar.activation(
                                c16[:, lo:hi],
                                ps[:, lo:hi],
                                mybir.ActivationFunctionType.Copy,
                            )
                        else:
                            nc.vector.tensor_scalar(
                                out=c16[:, lo:hi],
                                in0=ps[:, lo:hi],
                                scalar1=1.0,
                                scalar2=None,
                                op0=mybir.AluOpType.mult,
                            )
                        out_eng = (nc.sync, nc.scalar)[q % 2]
                        out_eng.dma_start(
                            out=c_view[mt][:, lo:hi], in_=c16[:, lo:hi]
                        )
                # outputs alternate the two HWDGE queues (idle once inputs
                # land); gpsimd/SWDGE is avoided — its end-of-kernel drain
                # costs ~4us
                if mt < MT - 1:
                    out_eng = (nc.sync, nc.scalar)[mt % 2]
                    out_eng.dma_start(out=c_view[mt], in_=c16)

    nc.finalize()  # bacc: regalloc + event-semaphore legalization
    return nc


def _get_nc():
    global _NC_CACHE
    if _NC_CACHE is None:
        _NC_CACHE = _build_bass()
    return _NC_CACHE


def _pack_inputs(b, G):
    b8 = np.asarray(b).astype(np.uint8)
    G8 = np.asarray(G).astype(np.uint8)
    # g: [p, s, n] with k = s*128 + p
    g_f8 = G8.reshape(KS, 128, N_CODE).transpose(1, 0, 2).astype(F8, order="C")
    bts = []
    for core in range(NCORES):
        sh = b8[core * M : (core + 1) * M]  # [M, K]
        # [p, mt, s, j]: k = s*128 + p, m = mt*128 + j
        bt = sh.reshape(MT, 128, KS, 128).transpose(3, 0, 2, 1)
        bts.append(bt.astype(F8, order="C"))
    return bts, g_f8


def kernel(b, G, trace=False, **run_kwargs):
    from concourse.bass_utils import run_bass_kernel_spmd

    nc = _get_nc()
    bts, g_f8 = _pack_inputs(b, G)
    in_maps = [{"bt": bts[i], "g": g_f8} for i in range(NCORES)]
    res = run_bass_kernel_spmd(
        nc, in_maps, core_ids=list(range(NCORES)), trace=trace, **run_kwargs
    )
    out = np.concatenate([res.results[i]["c"] for i in range(NCORES)], axis=0)
    out = (out & 1).astype(np.int32)
    if trace:
        kernel.last_results = res
    return out


kernel.last_results = None
